# revision 11
# baseline (speedup 1.0000x reference)
"""Trainium2 Bass kernel for the intra-batch point-cloud contrastive loss.

Math (matches the reference):
  feats   = features_in.reshape(C, M).T    (row-major reinterpret), M = B*N
  labels  = labels_in.reshape(-1)
  sel     = bernoulli(key 42, min(750/(count+1),1)[labels])   (host, jax CPU)
  nv      = feats / ||feats||
  dp      = exp(nv @ nv.T / TEMP), diagonal zeroed
  pos_i   = sum_{j sel, same class} dp_ij ; neg over different class
  loss    = mean over selected i of -log(pos/(pos+neg))

Only selected points matter (~3001 of 8192).  The selected points are
SORTED BY CLASS and each class is padded with zero-feature points to
2*SEG columns (SEG=384 -> M_pad=3072).  Rows are sharded over 8 cores
(SEG rows each, rolled so each core's own columns come first); each core
computes its row-block of the similarity matrix against ALL columns in
bf16, exponentiates, and reduces each row over the 8 column segments of
SEG.  Columns are class-sorted and the per-core roll is a multiple of
SEG, so every segment is class-pure: the 8 per-segment row sums ARE the
per-class sums.  The host maps segment -> class per core, subtracts the
exp(0)=1 contribution of the zero pads, and runs the tiny O(n_sel)
epilogue.

Per core and row chunk r (nL = SEG/128 chunks):
  mm1 (PE):  ps[512b:512b+512] = nv[:, rP:(r+1)P].T @ nv[:, cols]   bf16
  diag (PE): ps[rP:rP+128] += I.T @ (-1e9*I)   (kills the diagonal)
  exp (ACT): dp = exp(ps / TEMP) -> SBUF bf16  (two [128, 1536] instrs)
  sum (DVE): TT-add fold 384->192->96, tensor_reduce -> acc[128, 8] f32
No second matmul chain, no O(M^2) output traffic.
"""

import numpy as np

TEMP = 0.07
NUM_CLASSES = 4
N_CORES = 8
P = 128

_NEFF_CACHE = {}
_results = [None]


def _compute_sel(labels_flat):
    """Selection mask, bit-exact with the reference (jax threefry, key 42)."""
    import jax
    import jax.numpy as jnp

    cpu = jax.devices("cpu")[0]
    with jax.default_device(cpu):
        lab_j = jnp.asarray(labels_flat)
        counts = jnp.bincount(lab_j, length=NUM_CLASSES)
        keep_p = jnp.minimum(750.0 / (counts.astype(jnp.float32) + 1.0), 1.0)
        p = keep_p[lab_j]
        sel = jax.random.bernoulli(jax.random.key(42), p)
        return np.asarray(sel)


def _build_kernel(SEG):
    import concourse.bass as bass
    import concourse.mybir as mybir
    import concourse.tile as tile

    nL = SEG // P                 # row chunks per core
    M_pad = 8 * SEG
    HB = M_pad // 2               # bytes of columns per half (h0: nv0-2, h1: nvb)
    f32 = mybir.dt.float32
    bf16 = mybir.dt.bfloat16
    Exp = mybir.ActivationFunctionType.Exp
    add = mybir.AluOpType.add
    AX = mybir.AxisListType.X
    NB = HB // 512                # 512-col blocks per half (3 for SEG=384)

    nc = bass.Bass()
    nv_d = [
        nc.dram_tensor(f"nv{i}", [64, 512], bf16, kind="ExternalInput")
        for i in range(2 * NB)
    ]
    consts_d = nc.dram_tensor("consts", [P, 2 * P], bf16, kind="ExternalInput")
    acc_d = nc.dram_tensor("acc", [P, nL * 8], f32, kind="ExternalOutput")

    with tile.TileContext(nc) as tc:
        with (
            tc.tile_pool(name="singles", bufs=1) as singles,
            tc.tile_pool(name="dp_pool", bufs=2) as dp_pool,
            tc.tile_pool(name="t1_pool", bufs=2) as t1_pool,
            tc.tile_pool(name="t2_pool", bufs=2) as t2_pool,
            tc.tile_pool(name="ps_pool", bufs=2, space="PSUM") as ps_pool,
        ):
            nva = [
                singles.tile([64, 512], bf16, name=f"nva{i}")
                for i in range(2 * NB)
            ]
            consts = singles.tile([P, 2 * P], bf16)
            # Two HWDGE queues, each in first-use order: blocks 0-1 + consts
            # on sync, blocks 2-5 on scalar, so no matmul waits on a queue.
            nc.sync.dma_start(out=nva[0][:], in_=nv_d[0][:])
            nc.sync.dma_start(out=nva[1][:], in_=nv_d[1][:])
            nc.sync.dma_start(out=consts[:], in_=consts_d[:])
            nc.scalar.dma_start(out=nva[2][:], in_=nv_d[2][:])
            nc.scalar.dma_start(out=nva[3][:], in_=nv_d[3][:])
            nc.scalar.dma_start(out=nva[4][:], in_=nv_d[4][:])
            nc.scalar.dma_start(out=nva[5][:], in_=nv_d[5][:])
            eye = consts[:, 0:P]
            eyeneg = consts[:, P:2 * P]

            acc = singles.tile([P, nL, 8], f32)

            for r in range(nL):
                stat = nva[0][:, r * P:(r + 1) * P]
                dp = dp_pool.tile([P, 8, SEG], bf16)
                for h in range(2):
                    ps = ps_pool.tile([P, HB], f32)
                    for b in range(NB):
                        nc.tensor.matmul(
                            ps[:, 512 * b:512 * (b + 1)], stat, nva[NB * h + b],
                            start=True, stop=not (h == 0 and b == 0),
                        )
                    if h == 0:
                        # add -1e9 on the rolled diagonal (cols rP..rP+P of
                        # block 0) so exp maps it to exactly 0
                        nc.tensor.matmul(
                            ps[:, r * P:(r + 1) * P], eye, eyeneg,
                            start=False, stop=True,
                        )
                    if r == nL - 1 and h == 1:
                        # split the last exp so the reduce tail starts sooner
                        for q in range(2):
                            nc.scalar.activation(
                                dp[:, 4 + 2 * q:6 + 2 * q, :],
                                ps[:, 768 * q:768 * (q + 1)],
                                Exp, scale=float(1.0 / TEMP),
                            )
                    else:
                        nc.scalar.activation(
                            dp[:, 4 * h:4 * h + 4, :], ps[:],
                            Exp, scale=float(1.0 / TEMP),
                        )
                t1 = t1_pool.tile([P, 8, SEG // 2], bf16)
                t2 = t2_pool.tile([P, 8, SEG // 4], bf16)
                if r < nL - 1:
                    for h in range(2):
                        nc.vector.tensor_tensor(
                            t1[:, 4 * h:4 * h + 4, :],
                            dp[:, 4 * h:4 * h + 4, 0:SEG // 2],
                            dp[:, 4 * h:4 * h + 4, SEG // 2:SEG],
                            op=add,
                        )
                    nc.vector.tensor_tensor(
                        t2[:], t1[:, :, 0:SEG // 4], t1[:, :, SEG // 4:SEG // 2],
                        op=add,
                    )
                    nc.vector.tensor_reduce(acc[:, r, :], t2[:], axis=AX, op=add)
                else:
                    # last chunk: finish sub-slices independently so each
                    # reduce overlaps the remaining exp (shorter serial tail)
                    for sl in (slice(0, 4), slice(4, 6), slice(6, 8)):
                        nc.vector.tensor_tensor(
                            t1[:, sl, :],
                            dp[:, sl, 0:SEG // 2], dp[:, sl, SEG // 2:SEG],
                            op=add,
                        )
                        nc.vector.tensor_tensor(
                            t2[:, sl, :],
                            t1[:, sl, 0:SEG // 4], t1[:, sl, SEG // 4:SEG // 2],
                            op=add,
                        )
                        nc.vector.tensor_reduce(
                            acc[:, r, sl], t2[:, sl, :], axis=AX, op=add,
                        )

            nc.sync.dma_start(out=acc_d[:], in_=acc[:])

    _split_multi_waits(nc)
    return nc


def _split_multi_waits(nc):
    """Walrus in this toolchain accepts only one inline sync-wait per
    instruction.  Tile's kernel-tail drain aggregates one wait per live
    semaphore, so hoist all but the last wait onto same-engine nops."""
    import concourse.mybir as mybir

    for fn in nc.m.functions:
        for blk in fn.blocks:
            insts = list(blk.instructions)
            out = []
            for inst in insts:
                si = inst.sync_info
                waits = list(si.on_wait) if si is not None and si.on_wait else []
                if len(waits) > 1:
                    for w in waits[:-1]:
                        out.append(mybir.InstNoOp(
                            name=nc.get_next_instruction_name(),
                            engine=inst.engine,
                            bass_nofuse=True,
                            sync_info=mybir.SyncInfo(on_wait=[w], on_update=[]),
                        ))
                    si.on_wait = waits[-1:]
                out.append(inst)
            if len(out) != len(insts):
                blk.instructions = out


def _get_kernel(SEG):
    if SEG not in _NEFF_CACHE:
        _NEFF_CACHE[SEG] = _build_kernel(SEG)
    return _NEFF_CACHE[SEG]


def kernel(features_in, labels_in, _trace=False, _results=_results):
    import ml_dtypes
    from concourse.bass_utils import run_bass_kernel_spmd

    features_in = np.asarray(features_in, dtype=np.float32)
    B, C, N = features_in.shape
    M = B * N
    labels = np.asarray(labels_in).reshape(-1).astype(np.int64)

    fT = features_in.reshape(C, M)                      # [C, M] reinterpret
    sel = _compute_sel(labels)
    idx = np.nonzero(sel)[0]
    n_sel = int(idx.size)
    lab_sel = labels[idx]

    norms = np.sqrt(np.sum(fT * fT, axis=0, dtype=np.float32)).astype(np.float32)
    nvT = (fT / norms).astype(np.float32)

    # Sort selected points by class; pad each class block to 2*SEG columns.
    n_c = np.bincount(lab_sel, minlength=NUM_CLASSES)
    SEG = max(384, 128 * int(np.ceil(n_c.max() / 256.0)))
    CAP = 2 * SEG                 # per-class capacity
    M_pad = 8 * SEG
    HB = M_pad // 2
    NB = HB // 512

    order = np.argsort(lab_sel, kind="stable")
    G = np.zeros((64, M_pad), dtype=ml_dtypes.bfloat16)
    # position of each sorted point in the padded layout
    pos = np.concatenate(
        [np.arange(n_c[c]) + CAP * c for c in range(NUM_CLASSES)]
    )
    nv_sel = nvT[:, idx[order]].astype(ml_dtypes.bfloat16)
    G[:, pos] = nv_sel

    eye = np.eye(P, dtype=ml_dtypes.bfloat16)
    eyeneg = (np.eye(P, dtype=np.float32) * -1e9).astype(ml_dtypes.bfloat16)
    consts = np.concatenate([eye, eyeneg], axis=1)

    in_maps = []
    for k in range(N_CORES):
        nv_k = np.roll(G, -SEG * k, axis=1)
        m = {
            f"nv{i}": np.ascontiguousarray(nv_k[:, 512 * i:512 * (i + 1)])
            for i in range(2 * NB)
        }
        m["consts"] = consts
        in_maps.append(m)

    nc = _get_kernel(SEG)
    res = run_bass_kernel_spmd(nc, in_maps, core_ids=list(range(N_CORES)),
                               trace=_trace)
    _results[0] = res

    nL = SEG // P
    # acc[k][p, r*8+s]: row sum of point (SEG*k + P*r + p) over local col
    # segment s = global segment (s+k) % 8.
    S_glob = np.zeros((M_pad, 8), dtype=np.float64)
    for k in range(N_CORES):
        a = np.asarray(res.results[k]["acc"], dtype=np.float64)
        a = a.reshape(P, nL, 8).transpose(1, 0, 2).reshape(SEG, 8)
        S_glob[SEG * k:SEG * (k + 1), (np.arange(8) + k) % 8] = a

    S4 = S_glob.reshape(M_pad, NUM_CLASSES, 2).sum(axis=2)  # [M_pad, 4]
    pads = (CAP - n_c).astype(np.float64)                   # exp(0)=1 per pad
    Sreal = S4[pos] - pads[None, :]                         # [n_sel, 4] sorted
    lab_sorted = lab_sel[order]
    numer = Sreal[np.arange(n_sel), lab_sorted]
    denom = Sreal.sum(axis=1)
    per = -np.log(numer / denom)
    loss = np.float32(per.sum() / max(n_sel, 1))
    return np.asarray(loss, dtype=np.float32)


# revision 12
# speedup vs baseline: 1.0447x; 1.0447x over previous
"""Trainium2 Bass kernel for the intra-batch point-cloud contrastive loss.

Math (matches the reference):
  feats   = features_in.reshape(C, M).T    (row-major reinterpret), M = B*N
  labels  = labels_in.reshape(-1)
  sel     = bernoulli(key 42, min(750/(count+1),1)[labels])   (host, jax CPU)
  nv      = feats / ||feats||
  dp      = exp(nv @ nv.T / TEMP), diagonal zeroed
  pos_i   = sum_{j sel, same class} dp_ij ; neg over different class
  loss    = mean over selected i of -log(pos/(pos+neg))

Only selected points matter (~3001 of 8192).  The selected points are
SORTED BY CLASS and each class is padded with zero-feature points to
2*SEG columns (SEG=384 -> M_pad=3072).  Rows are sharded over 8 cores
(SEG rows each, rolled so each core's own columns come first); each core
computes its row-block of the similarity matrix against ALL columns in
bf16, exponentiates, and reduces each row over the 8 column segments of
SEG.  Columns are class-sorted and the per-core roll is a multiple of
SEG, so every segment is class-pure: the 8 per-segment row sums ARE the
per-class sums.  The host maps segment -> class per core, subtracts the
exp(0)=1 contribution of the zero pads, and runs the tiny O(n_sel)
epilogue.

Per core and row chunk r (nL = SEG/128 chunks):
  mm1 (PE):  ps[512b:512b+512] = nv[:, rP:(r+1)P].T @ nv[:, cols]   bf16
  diag (PE): ps[rP:rP+128] += I.T @ (-1e9*I)   (kills the diagonal)
  exp (ACT): dp = exp(ps / TEMP) -> SBUF bf16  (two [128, 1536] instrs)
  sum (DVE): TT-add fold 384->192->96, tensor_reduce -> acc[128, 8] f32
No second matmul chain, no O(M^2) output traffic.
"""

import numpy as np

TEMP = 0.07
NUM_CLASSES = 4
N_CORES = 8
P = 128

_NEFF_CACHE = {}
_results = [None]


def _compute_sel(labels_flat):
    """Selection mask, bit-exact with the reference (jax threefry, key 42)."""
    import jax
    import jax.numpy as jnp

    cpu = jax.devices("cpu")[0]
    with jax.default_device(cpu):
        lab_j = jnp.asarray(labels_flat)
        counts = jnp.bincount(lab_j, length=NUM_CLASSES)
        keep_p = jnp.minimum(750.0 / (counts.astype(jnp.float32) + 1.0), 1.0)
        p = keep_p[lab_j]
        sel = jax.random.bernoulli(jax.random.key(42), p)
        return np.asarray(sel)


def _build_kernel(SEG):
    import concourse.bass as bass
    import concourse.mybir as mybir
    import concourse.tile as tile

    nL = SEG // P                 # row chunks per core
    M_pad = 8 * SEG
    HB = M_pad // 2               # bytes of columns per half (h0: nv0-2, h1: nvb)
    f32 = mybir.dt.float32
    bf16 = mybir.dt.bfloat16
    Exp = mybir.ActivationFunctionType.Exp
    add = mybir.AluOpType.add
    AX = mybir.AxisListType.X
    NB = HB // 512                # 512-col blocks per half (3 for SEG=384)

    nc = bass.Bass()
    nv_d = [
        nc.dram_tensor(f"nv{i}", [64, 512], bf16, kind="ExternalInput")
        for i in range(2 * NB)
    ]
    consts_d = nc.dram_tensor("consts", [P, 2 * P], bf16, kind="ExternalInput")
    acc_d = nc.dram_tensor("acc", [P, nL * 8], f32, kind="ExternalOutput")

    with tile.TileContext(nc) as tc:
        with (
            tc.tile_pool(name="singles", bufs=1) as singles,
            tc.tile_pool(name="dp_pool", bufs=2) as dp_pool,
            tc.tile_pool(name="t1_pool", bufs=2) as t1_pool,
            tc.tile_pool(name="t2_pool", bufs=2) as t2_pool,
            tc.tile_pool(name="ps_pool", bufs=2, space="PSUM") as ps_pool,
        ):
            nva = [
                singles.tile([64, 512], bf16, name=f"nva{i}")
                for i in range(2 * NB)
            ]
            consts = singles.tile([P, 2 * P], bf16)
            # Single HWDGE queue in first-use order (concurrent queues were
            # measurably slower — the transfers share DMA fabric bandwidth).
            nc.sync.dma_start(out=nva[0][:], in_=nv_d[0][:])
            nc.sync.dma_start(out=nva[1][:], in_=nv_d[1][:])
            nc.sync.dma_start(out=nva[2][:], in_=nv_d[2][:])
            nc.sync.dma_start(out=consts[:], in_=consts_d[:])
            nc.sync.dma_start(out=nva[3][:], in_=nv_d[3][:])
            nc.sync.dma_start(out=nva[4][:], in_=nv_d[4][:])
            nc.sync.dma_start(out=nva[5][:], in_=nv_d[5][:])
            eye = consts[:, 0:P]
            eyeneg = consts[:, P:2 * P]

            acc = singles.tile([P, nL, 8], f32)

            for r in range(nL):
                stat = nva[0][:, r * P:(r + 1) * P]
                dp = dp_pool.tile([P, 8, SEG], bf16)
                for h in range(2):
                    ps = ps_pool.tile([P, HB], f32)
                    for b in range(NB):
                        nc.tensor.matmul(
                            ps[:, 512 * b:512 * (b + 1)], stat, nva[NB * h + b],
                            start=True, stop=not (h == 0 and b == 0),
                        )
                    if h == 0:
                        # add -1e9 on the rolled diagonal (cols rP..rP+P of
                        # block 0) so exp maps it to exactly 0
                        nc.tensor.matmul(
                            ps[:, r * P:(r + 1) * P], eye, eyeneg,
                            start=False, stop=True,
                        )
                    if r == nL - 1 and h == 1:
                        # split the last exp so the reduce tail starts sooner
                        for q in range(2):
                            nc.scalar.activation(
                                dp[:, 4 + 2 * q:6 + 2 * q, :],
                                ps[:, 768 * q:768 * (q + 1)],
                                Exp, scale=float(1.0 / TEMP),
                            )
                    else:
                        nc.scalar.activation(
                            dp[:, 4 * h:4 * h + 4, :], ps[:],
                            Exp, scale=float(1.0 / TEMP),
                        )
                t1 = t1_pool.tile([P, 8, SEG // 2], bf16)
                t2 = t2_pool.tile([P, 8, SEG // 4], bf16)
                if r < nL - 1:
                    for h in range(2):
                        nc.vector.tensor_tensor(
                            t1[:, 4 * h:4 * h + 4, :],
                            dp[:, 4 * h:4 * h + 4, 0:SEG // 2],
                            dp[:, 4 * h:4 * h + 4, SEG // 2:SEG],
                            op=add,
                        )
                    nc.vector.tensor_tensor(
                        t2[:], t1[:, :, 0:SEG // 4], t1[:, :, SEG // 4:SEG // 2],
                        op=add,
                    )
                    nc.vector.tensor_reduce(acc[:, r, :], t2[:], axis=AX, op=add)
                else:
                    # last chunk: finish sub-slices independently so each
                    # reduce overlaps the remaining exp (shorter serial tail)
                    for sl in (slice(0, 4), slice(4, 6), slice(6, 8)):
                        nc.vector.tensor_tensor(
                            t1[:, sl, :],
                            dp[:, sl, 0:SEG // 2], dp[:, sl, SEG // 2:SEG],
                            op=add,
                        )
                        nc.vector.tensor_tensor(
                            t2[:, sl, :],
                            t1[:, sl, 0:SEG // 4], t1[:, sl, SEG // 4:SEG // 2],
                            op=add,
                        )
                        nc.vector.tensor_reduce(
                            acc[:, r, sl], t2[:, sl, :], axis=AX, op=add,
                        )

            nc.sync.dma_start(out=acc_d[:], in_=acc[:])

    _split_multi_waits(nc)
    return nc


def _split_multi_waits(nc):
    """Walrus in this toolchain accepts only one inline sync-wait per
    instruction.  Tile's kernel-tail drain aggregates one wait per live
    semaphore, so hoist all but the last wait onto same-engine nops."""
    import concourse.mybir as mybir

    for fn in nc.m.functions:
        for blk in fn.blocks:
            insts = list(blk.instructions)
            out = []
            for inst in insts:
                si = inst.sync_info
                waits = list(si.on_wait) if si is not None and si.on_wait else []
                if len(waits) > 1:
                    for w in waits[:-1]:
                        out.append(mybir.InstNoOp(
                            name=nc.get_next_instruction_name(),
                            engine=inst.engine,
                            bass_nofuse=True,
                            sync_info=mybir.SyncInfo(on_wait=[w], on_update=[]),
                        ))
                    si.on_wait = waits[-1:]
                out.append(inst)
            if len(out) != len(insts):
                blk.instructions = out


def _get_kernel(SEG):
    if SEG not in _NEFF_CACHE:
        _NEFF_CACHE[SEG] = _build_kernel(SEG)
    return _NEFF_CACHE[SEG]


def kernel(features_in, labels_in, _trace=False, _results=_results):
    import ml_dtypes
    from concourse.bass_utils import run_bass_kernel_spmd

    features_in = np.asarray(features_in, dtype=np.float32)
    B, C, N = features_in.shape
    M = B * N
    labels = np.asarray(labels_in).reshape(-1).astype(np.int64)

    fT = features_in.reshape(C, M)                      # [C, M] reinterpret
    sel = _compute_sel(labels)
    idx = np.nonzero(sel)[0]
    n_sel = int(idx.size)
    lab_sel = labels[idx]

    norms = np.sqrt(np.sum(fT * fT, axis=0, dtype=np.float32)).astype(np.float32)
    nvT = (fT / norms).astype(np.float32)

    # Sort selected points by class; pad each class block to 2*SEG columns.
    n_c = np.bincount(lab_sel, minlength=NUM_CLASSES)
    SEG = max(384, 128 * int(np.ceil(n_c.max() / 256.0)))
    CAP = 2 * SEG                 # per-class capacity
    M_pad = 8 * SEG
    HB = M_pad // 2
    NB = HB // 512

    order = np.argsort(lab_sel, kind="stable")
    G = np.zeros((64, M_pad), dtype=ml_dtypes.bfloat16)
    # position of each sorted point in the padded layout
    pos = np.concatenate(
        [np.arange(n_c[c]) + CAP * c for c in range(NUM_CLASSES)]
    )
    nv_sel = nvT[:, idx[order]].astype(ml_dtypes.bfloat16)
    G[:, pos] = nv_sel

    eye = np.eye(P, dtype=ml_dtypes.bfloat16)
    eyeneg = (np.eye(P, dtype=np.float32) * -1e9).astype(ml_dtypes.bfloat16)
    consts = np.concatenate([eye, eyeneg], axis=1)

    in_maps = []
    for k in range(N_CORES):
        nv_k = np.roll(G, -SEG * k, axis=1)
        m = {
            f"nv{i}": np.ascontiguousarray(nv_k[:, 512 * i:512 * (i + 1)])
            for i in range(2 * NB)
        }
        m["consts"] = consts
        in_maps.append(m)

    nc = _get_kernel(SEG)
    res = run_bass_kernel_spmd(nc, in_maps, core_ids=list(range(N_CORES)),
                               trace=_trace)
    _results[0] = res

    nL = SEG // P
    # acc[k][p, r*8+s]: row sum of point (SEG*k + P*r + p) over local col
    # segment s = global segment (s+k) % 8.
    S_glob = np.zeros((M_pad, 8), dtype=np.float64)
    for k in range(N_CORES):
        a = np.asarray(res.results[k]["acc"], dtype=np.float64)
        a = a.reshape(P, nL, 8).transpose(1, 0, 2).reshape(SEG, 8)
        S_glob[SEG * k:SEG * (k + 1), (np.arange(8) + k) % 8] = a

    S4 = S_glob.reshape(M_pad, NUM_CLASSES, 2).sum(axis=2)  # [M_pad, 4]
    pads = (CAP - n_c).astype(np.float64)                   # exp(0)=1 per pad
    Sreal = S4[pos] - pads[None, :]                         # [n_sel, 4] sorted
    lab_sorted = lab_sel[order]
    numer = Sreal[np.arange(n_sel), lab_sorted]
    denom = Sreal.sum(axis=1)
    per = -np.log(numer / denom)
    loss = np.float32(per.sum() / max(n_sel, 1))
    return np.asarray(loss, dtype=np.float32)
